# revision 17
# baseline (speedup 1.0000x reference)
"""DeepSwarmLDA Trainium2 kernel (fp16 streaming version).

Math: reference computes
    Xg        = X[:, gene_idx]                        [B, L, G]
    ldas_out  = einsum('blg,lcg->bcl', Xg, lda_W) + lda_b.T
    h         = gelu(ldas_out @ W0.T + b0)            [B, C, 100]
    h         = gelu(h @ W1.T + b1)                   [B, C, 10]
    out       = h @ W2.T + b2                         [B, C, 1]

Everything up to the first gelu is linear in X, so the gather, the per-LDA
classifiers and W0 fold (on host, in float64) into one dense matrix:
    Mfold[n, (c,j)] = sum_{l,g} [gene_idx[l,g]==n] * lda_W[l,c,g] * W0[j,l]
    bias0[(c,j)]    = sum_l lda_b[l,c] * W0[j,l] + b0[j]
giving  h0 = gelu(X @ Mfold + bias0).  The remaining layers act per-c and
fold into block-diagonal matrices W1blk [(c,j),(c,k)] and W2blk [(c,k), c].

Device computes everything transposed (batch on the matmul free axis) so the
contraction dim always sits on SBUF partitions and no transposes are needed:
    h0T[(c,j), b] = gelu(Mfold_tile.T @ XT)           (bias0 via ones-row)
    h1T[(c,k), b] = gelu(W1blk.T @ h0T)               (b1 via sentinel chan)
    outT[c, b]    = W2blk.T @ h1T                     (b2 via sentinel chan)

All biases ride inside the matmuls using pad space that already exists:
  - X row NG (=2000, inside the 2048 pad) is a constant 1/SX row and
    Mfold row NG holds bias0*SM, so X@M picks up bias0 exactly.
  - Mfold pad column M0 (=500) makes h0_pre[500,:] == 8.0 exactly;
    gelu(8) == 8 to fp64 precision, so W1blk row 500 = b1/8 adds b1, and
    the same sentinel trick through W1blk col 50 adds b2 via W2blk row 50.

Matmul operands are fp16: Mfold scaled by SM=32, X by SX=1/32 (exact powers
of two, product unscaled) so every fp16 value sits far from the subnormal
range even though Mfold entries are ~5e-4. fp16 keeps tf32-level accuracy
(host sim: 3.4e-4 absmax-rel) at half the fp32 DMA bytes and full PE rate.

Sharding over 8 cores: batch split 4 ways (256 rows each) x C split 2 ways
(c 0-4 / c 5-9, so the aggregation MLP stays core-local).
"""

import numpy as np

import concourse.bass as bass
import concourse.mybir as mybir
from concourse.tile import TileContext
from concourse.tile_rust import add_dep_helper
from concourse.bass_utils import run_bass_kernel_spmd

# Problem shape (hardcoded per contract; kernel.py must be self-contained).
B, NG, L, G, C = 1024, 2000, 1000, 50, 10
J0, J1 = 100, 10

N_CORES = 8
PB, QC = 4, 2              # batch split x c split
BS = B // PB               # 256 batch rows per core
CS = C // QC               # 5 classes per core
KP = 2048                  # NG padded to 16 k-tiles of 128
KT = KP // 128             # 16
M0 = CS * J0               # 500 h0 channels per core
M0P = 512                  # padded -> 4 m-tiles of 128
MT = M0P // 128            # 4
M1 = CS * J1               # 50 h1 channels per core
M1P = 64                   # padded
F32 = mybir.dt.float32

# Operand scaling (exact powers of two; product is unscaled).
SM, SX = 32.0, 1.0 / 32.0
SENT = 8.0                 # sentinel: gelu(8) == 8 exactly in fp64/fp16

# DMA chunk sizes (k-tiles per DMA). Small first chunks so the PE starts
# early; small last chunks so the epilogue's final dependency lands early.
CHUNKS = [1, 1, 2, 2, 2, 2, 2, 2, 1, 1]
W12W = MT * M1P + CS       # 261 packed fp16 W1blk+W2blk columns

MM_DTYPES = {
    "f32": mybir.dt.float32,
    "f32r": mybir.dt.float32r,
    "bf16": mybir.dt.bfloat16,
    "f16": mybir.dt.float16,
}


def _build_program(act=None, mm="f16"):
    act = act if act is not None else mybir.ActivationFunctionType.Gelu
    mmdt = MM_DTYPES[mm]
    nc = bass.Bass()
    # DRAM layouts are pre-swizzled on host so every DMA is contiguous.
    # M-operand and X-operand interleave per k-tile in ONE tensor so each
    # chunk arrives via a single DMA (a matmul may depend on at most one
    # in-flight transfer):
    #   mx [128, KT, M0P+BS]  mx[p, k, :M0P] = Mfold[k*128+p, colshard]*SM
    #                         mx[p, k, M0P:] = X[bshard, k*128+p]*SX
    # w12 [128, W12W] fp16:   cols 0:256  W1blk swizzled
    #                         (col m*64+o = W1blk[m*128+p, o])
    #                         cols 256:261 W2blk (rows 0:64)
    mx = nc.declare_dram_parameter("mx", [128, KT, M0P + BS], mmdt, isOutput=False)
    w12 = nc.declare_dram_parameter("w12", [128, W12W], mmdt, isOutput=False)
    out = nc.declare_dram_parameter("out", [CS, BS], F32, isOutput=True)

    with TileContext(nc) as tc:
        with (
            tc.tile_pool(name="big", bufs=1) as big,
            tc.tile_pool(name="small", bufs=1) as small,
            tc.tile_pool(name="psum", bufs=1, space="PSUM") as psum,
        ):
            sync_targets = []     # observed by SP nops BEFORE the out-DMA
            late_targets = []     # observed after

            # Big operand chunks (one tile per chunk so the PE starts as
            # soon as a chunk lands). ALL triggers go on ONE ring: the 16
            # DMA queues are shared and work-conserving, so concurrent
            # triggers from different rings interleave their descriptors
            # and every chunk completes later than its wire-serial
            # position. Serial descriptor generation (~0.7us per chunk)
            # stays well ahead of the ~1.3us/chunk wire time.
            mx_ch = []   # per k-tile: (chunk tile, index within chunk)
            w12_sb = small.tile([128, W12W], mmdt, name="w12_sb")
            c0 = 0
            for ci, csz in enumerate(CHUNKS):
                t = big.tile([128, csz, M0P + BS], mmdt, name=f"mx_ch{ci}")
                sync_targets.append(
                    nc.sync.dma_start(out=t[:], in_=mx[:, c0 : c0 + csz])
                )
                for j in range(csz):
                    mx_ch.append((t, j))
                c0 += csz
                if ci == len(CHUNKS) - 3:
                    # w12 rides late in the trigger train (nothing reads
                    # it until h1) but before the final chunks so its
                    # data is guaranteed in before the PE touch.
                    sync_targets.append(
                        nc.sync.dma_start(out=w12_sb[:], in_=w12[:])
                    )
            assert c0 == KT

            # ACT touch: observes the w12 lane on ACT early AND pre-warms
            # the gelu table (a lazy first-use table load costs ~1.3us on
            # the critical tail).
            t_sb = small.tile([128, 1], F32, name="t_sb")
            nc.scalar.activation(t_sb[:], w12_sb[:, 0:1], act)

            # PE warmup: the PE runs at 0.65/1.2 GHz until it has been
            # continuously busy for ~3us, only then at 2.4 GHz. Dep-free
            # dummy matmuls on (uninitialized) SBUF ramp it to full clock
            # during the trigger+first-transfer dead time, so the real
            # stream starts at full rate. Results land in a scratch PSUM
            # tile; every start/stop pair precedes the first real matmul
            # in PE program order, so bank clears cannot hurt real data.
            warm_sb = small.tile([128, 256], mmdt, name="warm_sb")
            nc.vector.memset(warm_sb[:], 0)
            warm_ps = psum.tile([128, 256], F32, name="warm_ps")

            def pe_warm(n):
                for _ in range(n):
                    nc.tensor.matmul(
                        warm_ps[:],
                        lhsT=warm_sb[:, 0:128],
                        rhs=warm_sb[:],
                        start=True,
                        stop=True,
                    )

            pe_warm(24)

            # h0T = Mfold.T @ XT accumulated over 16 k-tiles, 4 m-tiles.
            # One PSUM tile per m-tile: matmul start=True clears the whole
            # PSUM bank, so accumulation regions must not share banks.
            h0_ps = [
                psum.tile([128, BS], F32, name=f"h0_ps{m}") for m in range(MT)
            ]
            for k in range(KT):
                t, j = mx_ch[k]
                ch = t[:, j]
                for m in range(MT):
                    nc.tensor.matmul(
                        h0_ps[m][:],
                        lhsT=ch[:, m * 128 : (m + 1) * 128],
                        rhs=ch[:, M0P:],
                        start=(k == 0),
                        stop=(k == KT - 1),
                    )

            # PE touch: observe the w12 lane between the big matmuls and
            # the w12-consuming h1 matmuls (keeps those at one new wait
            # each without ever stalling the in-order PE queue).
            t_ps = psum.tile([1, 1], F32, name="t_ps")
            nc.tensor.matmul(
                t_ps[:], lhsT=w12_sb[:, 0:1], rhs=w12_sb[:, 0:1],
                start=True, stop=True,
            )
            # Keep the PE clock hot through the gelu phase so the h1
            # matmuls don't pay the mid-p-state rate.
            pe_warm(10)

            # gelu(h0), PSUM -> SBUF fp16, one slab per m-tile so each
            # pipelines against the PE's final k-tile matmuls.
            h0_sb = small.tile([128, MT, BS], mmdt, name="h0_sb")
            for m in range(MT):
                nc.scalar.activation(h0_sb[:, m], h0_ps[m][:], act)

            # h1T = W1blk.T @ h0T (contraction = 4 m-tiles of 128).
            h1_ps = psum.tile([M1P, BS], F32, name="h1_ps")
            for m in range(MT):
                nc.tensor.matmul(
                    h1_ps[:],
                    lhsT=w12_sb[:, m * M1P : (m + 1) * M1P],
                    rhs=h0_sb[:, m],
                    start=(m == 0),
                    stop=(m == MT - 1),
                )
            pe_warm(3)
            h1_sb = small.tile([M1P, BS], mmdt, name="h1_sb")
            nc.scalar.activation(h1_sb[:], h1_ps[:], act)

            # outT = W2blk.T @ h1T  (b2 rides on sentinel channel 50).
            o_ps = psum.tile([CS, BS], F32, name="o_ps")
            sync_targets.append(nc.tensor.matmul(
                o_ps[:],
                lhsT=w12_sb[0:M1P, MT * M1P : MT * M1P + CS],
                rhs=h1_sb[:],
                start=True,
                stop=True,
            ))
            o_sb = small.tile([CS, BS], F32, name="o_sb")
            late_targets.append(nc.vector.tensor_copy(out=o_sb[:], in_=o_ps[:]))

            # The kernel-tail drain puts a wait on every proc SP has not
            # observed, and its encoding holds only a few waits. Chain SP
            # NOPs, one sync dep each, so SP observes every DMA lane and
            # engine tick incrementally and the drain has nothing left.
            # Observing all input-DMA lanes BEFORE issuing the output DMA
            # also elides the out-DMA's same-lane ordering wait (HWDGE
            # waits execute on the issuing sequencer), keeping it at the
            # one-wait encoding limit regardless of input DMA count.
            for t in sync_targets:
                nop = nc.sync.nop()
                add_dep_helper(
                    nop.ins, t.ins, sync=True, reason="spread drain waits"
                )

            # Out-DMA on the gpsimd ring: it carries no other DMAs, so the
            # only wait is the o_sb producer tick.
            late_targets.append(nc.gpsimd.dma_start(
                out=out[:], in_=o_sb[:], single_packet=True
            ))
            for t in late_targets:
                nop = nc.sync.nop()
                add_dep_helper(
                    nop.ins, t.ins, sync=True, reason="spread drain waits"
                )

    return nc


def _fold_weights(gene_idx, lda_W, lda_b, W0, b0):
    """Fold gather + per-LDA linear + W0 into Mfold [NG, C, J0] and
    bias0 [C, J0], computed in float64."""
    lda_W64 = lda_W.astype(np.float64)
    W064 = W0.astype(np.float64)
    # A[l, n, c] = sum_g [gene_idx[l,g]==n] * lda_W[l,c,g]
    A = np.zeros((L, NG, C), dtype=np.float64)
    l_rep = np.repeat(np.arange(L), G)
    np.add.at(A, (l_rep, gene_idx.ravel()), lda_W64.transpose(0, 2, 1).reshape(L * G, C))
    # Mfold[n, c, j] = sum_l A[l, n, c] * W0[j, l]
    Mfold = (W064 @ A.reshape(L, NG * C)).reshape(J0, NG, C).transpose(1, 2, 0)
    bias0 = np.einsum("lc,jl->cj", lda_b.astype(np.float64), W064) + b0.astype(
        np.float64
    )
    return Mfold, bias0


_prog_cache = {}


def _get_program(act=None, mm="f16"):
    key = ("nc", act, mm)
    if key not in _prog_cache:
        _prog_cache[key] = _build_program(act, mm)
    return _prog_cache[key]


def _round_tf32(a):
    """Round fp32 array to the TF32 grid (10-bit mantissa, RNE)."""
    u = np.ascontiguousarray(a, dtype=np.float32).view(np.uint32)
    lsb = (u >> 13) & np.uint32(1)
    u2 = (u + np.uint32(0x0FFF) + lsb) & np.uint32(0xFFFFE000)
    return u2.view(np.float32)


def _mm_convert(a, mm):
    if mm == "f32":
        return np.asarray(a, dtype=np.float32)
    if mm == "f32r":
        return _round_tf32(np.asarray(a, dtype=np.float32))
    if mm == "bf16":
        import ml_dtypes

        return np.asarray(a, dtype=np.float32).astype(ml_dtypes.bfloat16)
    if mm == "f16":
        return np.asarray(a, dtype=np.float32).astype(np.float16)
    raise ValueError(mm)


def _prepare_in_maps(X, gene_idx, lda_W, lda_b, W0, b0, W1, b1, W2, b2, mm="f16"):
    X = np.asarray(X, dtype=np.float32)
    gene_idx = np.asarray(gene_idx)
    lda_W = np.asarray(lda_W, dtype=np.float32)
    lda_b = np.asarray(lda_b, dtype=np.float32)
    W0 = np.asarray(W0, dtype=np.float32)
    b0 = np.asarray(b0, dtype=np.float32)
    W1 = np.asarray(W1, dtype=np.float32)
    b1 = np.asarray(b1, dtype=np.float32)
    W2 = np.asarray(W2, dtype=np.float32)
    b2 = np.asarray(b2, dtype=np.float32)

    Mfold, bias0 = _fold_weights(gene_idx, lda_W, lda_b, W0, b0)

    # Per-C-half M shards: Mfold columns c-major flattened, scaled by SM,
    # bias0 in row NG (met by X's 1/SX ones-row), sentinel col M0.
    mf_maps, w12_maps = [], []
    for ch in range(QC):
        cs = slice(ch * CS, (ch + 1) * CS)
        mcols = Mfold[:, cs, :].reshape(NG, M0)
        mpad = np.zeros((KP, M0P), dtype=np.float64)
        mpad[:NG, :M0] = mcols * SM
        mpad[NG, :M0] = bias0[cs, :].reshape(M0) * SM
        mpad[NG, M0] = SENT * SM  # with X's 1/SX row: h0_pre[M0,:] == SENT
        mf_maps.append(np.ascontiguousarray(
            mpad.reshape(KT, 128, M0P).transpose(1, 0, 2)))

        # Packed fp16 W1blk/W2blk [128, W12W].
        w1blk = np.zeros((M0P, M1P), dtype=np.float64)
        for c in range(CS):
            w1blk[c * J0 : (c + 1) * J0, c * J1 : (c + 1) * J1] = W1.T
        # b1 rides on the h0 sentinel channel (value SENT after gelu).
        for c in range(CS):
            w1blk[M0, c * J1 : (c + 1) * J1] = b1 / SENT
        # b2 hook: make h1 channel M1 a sentinel too.
        w1blk[M0, M1] = 1.0
        w2blk = np.zeros((M1P, CS), dtype=np.float64)
        for c in range(CS):
            w2blk[c * J1 : (c + 1) * J1, c] = W2[0]
        w2blk[M1, :] = b2[0] / SENT
        w_arr = np.zeros((128, W12W), dtype=np.float64)
        w_arr[:, : MT * M1P] = (
            w1blk.reshape(MT, 128, M1P).transpose(1, 0, 2).reshape(128, MT * M1P)
        )
        w_arr[:M1P, MT * M1P :] = w2blk
        w12_maps.append(w_arr)

    # Batch shards, transposed + 1/SX ones-row + swizzled to [128, KT, BS].
    xt_maps = []
    for bq in range(PB):
        xs = X[bq * BS : (bq + 1) * BS, :]  # [BS, NG]
        xpad = np.zeros((KP, BS), dtype=np.float64)
        xpad[:NG, :] = xs.T * SX
        xpad[NG, :] = SX
        xt_maps.append(np.ascontiguousarray(
            xpad.reshape(KT, 128, BS).transpose(1, 0, 2)))

    in_maps = []
    for core in range(N_CORES):
        bq, ch = core % PB, core // PB
        mxa = np.concatenate([mf_maps[ch], xt_maps[bq]], axis=2)
        in_maps.append({
            "mx": _mm_convert(np.ascontiguousarray(mxa), mm),
            "w12": _mm_convert(w12_maps[ch], mm),
        })
    return in_maps


def _assemble(core_outs):
    out = np.empty((B, C, 1), dtype=np.float32)
    for core in range(N_CORES):
        bq, ch = core % PB, core // PB
        o = core_outs[core]  # [CS, BS]
        out[bq * BS : (bq + 1) * BS, ch * CS : (ch + 1) * CS, 0] = o.T
    return out


MM_MODE = "f16"


def kernel(X, gene_idx, lda_W, lda_b, W0, b0, W1, b1, W2, b2, _trace=False,
           _mm=None):
    mm = _mm or MM_MODE
    in_maps = _prepare_in_maps(
        X, gene_idx, lda_W, lda_b, W0, b0, W1, b1, W2, b2, mm=mm
    )
    nc = _get_program(mm=mm)
    res = run_bass_kernel_spmd(
        nc, in_maps, core_ids=list(range(N_CORES)), trace=_trace
    )
    out = _assemble([res.results[c]["out"] for c in range(N_CORES)])
    if _trace:
        return out, res
    return out


# revision 19
# speedup vs baseline: 1.0561x; 1.0561x over previous
"""DeepSwarmLDA Trainium2 kernel (fp16 streaming version).

Math: reference computes
    Xg        = X[:, gene_idx]                        [B, L, G]
    ldas_out  = einsum('blg,lcg->bcl', Xg, lda_W) + lda_b.T
    h         = gelu(ldas_out @ W0.T + b0)            [B, C, 100]
    h         = gelu(h @ W1.T + b1)                   [B, C, 10]
    out       = h @ W2.T + b2                         [B, C, 1]

Everything up to the first gelu is linear in X, so the gather, the per-LDA
classifiers and W0 fold (on host, in float64) into one dense matrix:
    Mfold[n, (c,j)] = sum_{l,g} [gene_idx[l,g]==n] * lda_W[l,c,g] * W0[j,l]
    bias0[(c,j)]    = sum_l lda_b[l,c] * W0[j,l] + b0[j]
giving  h0 = gelu(X @ Mfold + bias0).  The remaining layers act per-c and
fold into block-diagonal matrices W1blk [(c,j),(c,k)] and W2blk [(c,k), c].

Device computes everything transposed (batch on the matmul free axis) so the
contraction dim always sits on SBUF partitions and no transposes are needed:
    h0T[(c,j), b] = gelu(Mfold_tile.T @ XT)           (bias0 via ones-row)
    h1T[(c,k), b] = gelu(W1blk.T @ h0T)               (b1 via sentinel chan)
    outT[c, b]    = W2blk.T @ h1T                     (b2 via sentinel chan)

All biases ride inside the matmuls using pad space that already exists:
  - X row NG (=2000, inside the 2048 pad) is a constant 1/SX row and
    Mfold row NG holds bias0*SM, so X@M picks up bias0 exactly.
  - Mfold pad column M0 (=500) makes h0_pre[500,:] == 8.0 exactly;
    gelu(8) == 8 to fp64 precision, so W1blk row 500 = b1/8 adds b1, and
    the same sentinel trick through W1blk col 50 adds b2 via W2blk row 50.

Matmul operands are fp16: Mfold scaled by SM=32, X by SX=1/32 (exact powers
of two, product unscaled) so every fp16 value sits far from the subnormal
range even though Mfold entries are ~5e-4. fp16 keeps tf32-level accuracy
(host sim: 3.4e-4 absmax-rel) at half the fp32 DMA bytes and full PE rate.

Sharding over 8 cores: batch split 4 ways (256 rows each) x C split 2 ways
(c 0-4 / c 5-9, so the aggregation MLP stays core-local).
"""

import numpy as np

import concourse.bass as bass
import concourse.mybir as mybir
from concourse.tile import TileContext
from concourse.tile_rust import add_dep_helper
from concourse.bass_utils import run_bass_kernel_spmd

# Problem shape (hardcoded per contract; kernel.py must be self-contained).
B, NG, L, G, C = 1024, 2000, 1000, 50, 10
J0, J1 = 100, 10

N_CORES = 8
PB, QC = 4, 2              # batch split x c split
BS = B // PB               # 256 batch rows per core
CS = C // QC               # 5 classes per core
KP = 2048                  # NG padded to 16 k-tiles of 128
KT = KP // 128             # 16
M0 = CS * J0               # 500 h0 channels per core
M0P = 512                  # padded -> 4 m-tiles of 128
MT = M0P // 128            # 4
M1 = CS * J1               # 50 h1 channels per core
M1P = 64                   # padded
F32 = mybir.dt.float32

# Operand scaling (exact powers of two; product is unscaled).
SM, SX = 32.0, 1.0 / 32.0
SENT = 8.0                 # sentinel: gelu(8) == 8 exactly in fp64/fp16

# DMA chunk sizes (k-tiles per DMA). Small first chunks so the PE starts
# early; small last chunks so the epilogue's final dependency lands early.
CHUNKS = [1, 1, 2, 2, 2, 2, 2, 2, 1, 1]
W12W = MT * M1P + CS       # 261 packed fp16 W1blk+W2blk columns

MM_DTYPES = {
    "f32": mybir.dt.float32,
    "f32r": mybir.dt.float32r,
    "bf16": mybir.dt.bfloat16,
    "f16": mybir.dt.float16,
}


def _build_program(act=None, mm="f16"):
    act = act if act is not None else mybir.ActivationFunctionType.Gelu
    mmdt = MM_DTYPES[mm]
    nc = bass.Bass()
    # DRAM layouts are pre-swizzled on host so every DMA is contiguous.
    # M-operand and X-operand interleave per k-tile in ONE tensor so each
    # chunk arrives via a single DMA (a matmul may depend on at most one
    # in-flight transfer):
    #   mx [128, KT, M0P+BS]  mx[p, k, :M0P] = Mfold[k*128+p, colshard]*SM
    #                         mx[p, k, M0P:] = X[bshard, k*128+p]*SX
    # w12 [128, W12W] fp16:   cols 0:256  W1blk swizzled
    #                         (col m*64+o = W1blk[m*128+p, o])
    #                         cols 256:261 W2blk (rows 0:64)
    mx = nc.declare_dram_parameter("mx", [128, KT, M0P + BS], mmdt, isOutput=False)
    w12 = nc.declare_dram_parameter("w12", [128, W12W], mmdt, isOutput=False)
    out = nc.declare_dram_parameter("out", [CS, BS], F32, isOutput=True)

    with TileContext(nc) as tc:
        with (
            tc.tile_pool(name="big", bufs=1) as big,
            tc.tile_pool(name="small", bufs=1) as small,
            tc.tile_pool(name="psum", bufs=1, space="PSUM") as psum,
        ):
            sync_targets = []     # observed by SP nops BEFORE the out-DMA
            late_targets = []     # observed after

            # Big operand chunks (one tile per chunk so the PE starts as
            # soon as a chunk lands). ALL triggers go on ONE ring: the 16
            # DMA queues are shared and work-conserving, so concurrent
            # triggers from different rings interleave their descriptors
            # and every chunk completes later than its wire-serial
            # position. Serial descriptor generation (~0.7us per chunk)
            # stays well ahead of the ~1.3us/chunk wire time.
            # w12 goes FIRST on the scalar ring: its 67KB interleaves with
            # chunk 0/1 descriptors (PE is still warming up then) and lands
            # early enough for the ACT gelu-table prewarm to finish long
            # before the first real gelu.
            w12_sb = small.tile([128, W12W], mmdt, name="w12_sb")
            sync_targets.append(nc.scalar.dma_start(out=w12_sb[:], in_=w12[:]))

            mx_ch = []   # per k-tile: (chunk tile, index within chunk)
            c0 = 0
            for ci, csz in enumerate(CHUNKS):
                t = big.tile([128, csz, M0P + BS], mmdt, name=f"mx_ch{ci}")
                sync_targets.append(
                    nc.sync.dma_start(out=t[:], in_=mx[:, c0 : c0 + csz])
                )
                for j in range(csz):
                    mx_ch.append((t, j))
                c0 += csz
            assert c0 == KT

            # ACT touch: observes the w12 lane on ACT early AND pre-warms
            # the gelu table (a lazy first-use table load costs ~1.3us on
            # the critical tail).
            t_sb = small.tile([128, 1], F32, name="t_sb")
            nc.scalar.activation(t_sb[:], w12_sb[:, 0:1], act)

            # PE warmup: the PE runs at 0.65/1.2 GHz until it has been
            # continuously busy for ~3us, only then at 2.4 GHz. Dep-free
            # dummy matmuls on (uninitialized) SBUF ramp it to full clock
            # during the trigger+first-transfer dead time, so the real
            # stream starts at full rate. Results land in a scratch PSUM
            # tile; every start/stop pair precedes the first real matmul
            # in PE program order, so bank clears cannot hurt real data.
            warm_sb = small.tile([128, 256], mmdt, name="warm_sb")
            nc.vector.memset(warm_sb[:], 0)
            warm_ps = psum.tile([128, 256], F32, name="warm_ps")

            def pe_warm(n):
                for _ in range(n):
                    nc.tensor.matmul(
                        warm_ps[:],
                        lhsT=warm_sb[:, 0:128],
                        rhs=warm_sb[:],
                        start=True,
                        stop=True,
                    )

            pe_warm(24)

            # h0T = Mfold.T @ XT accumulated over 16 k-tiles, 4 m-tiles.
            # One PSUM tile per m-tile: matmul start=True clears the whole
            # PSUM bank, so accumulation regions must not share banks.
            h0_ps = [
                psum.tile([128, BS], F32, name=f"h0_ps{m}") for m in range(MT)
            ]
            for k in range(KT):
                t, j = mx_ch[k]
                ch = t[:, j]
                for m in range(MT):
                    nc.tensor.matmul(
                        h0_ps[m][:],
                        lhsT=ch[:, m * 128 : (m + 1) * 128],
                        rhs=ch[:, M0P:],
                        start=(k == 0),
                        stop=(k == KT - 1),
                    )

            # PE touch: observe the w12 lane between the big matmuls and
            # the w12-consuming h1 matmuls (keeps those at one new wait
            # each without ever stalling the in-order PE queue).
            t_ps = psum.tile([1, 1], F32, name="t_ps")
            nc.tensor.matmul(
                t_ps[:], lhsT=w12_sb[:, 0:1], rhs=w12_sb[:, 0:1],
                start=True, stop=True,
            )

            # gelu(h0), PSUM -> SBUF fp16, one slab per m-tile so each
            # pipelines against the PE's final k-tile matmuls.
            h0_sb = small.tile([128, MT, BS], mmdt, name="h0_sb")
            for m in range(MT):
                nc.scalar.activation(h0_sb[:, m], h0_ps[m][:], act)

            # h1T = W1blk.T @ h0T (contraction = 4 m-tiles of 128).
            h1_ps = psum.tile([M1P, BS], F32, name="h1_ps")
            for m in range(MT):
                nc.tensor.matmul(
                    h1_ps[:],
                    lhsT=w12_sb[:, m * M1P : (m + 1) * M1P],
                    rhs=h0_sb[:, m],
                    start=(m == 0),
                    stop=(m == MT - 1),
                )
            h1_sb = small.tile([M1P, BS], mmdt, name="h1_sb")
            nc.scalar.activation(h1_sb[:], h1_ps[:], act)

            # outT = W2blk.T @ h1T  (b2 rides on sentinel channel 50).
            o_ps = psum.tile([CS, BS], F32, name="o_ps")
            sync_targets.append(nc.tensor.matmul(
                o_ps[:],
                lhsT=w12_sb[0:M1P, MT * M1P : MT * M1P + CS],
                rhs=h1_sb[:],
                start=True,
                stop=True,
            ))
            o_sb = small.tile([CS, BS], F32, name="o_sb")
            late_targets.append(nc.vector.tensor_copy(out=o_sb[:], in_=o_ps[:]))

            # The kernel-tail drain puts a wait on every proc SP has not
            # observed, and its encoding holds only a few waits. Chain SP
            # NOPs, one sync dep each, so SP observes every DMA lane and
            # engine tick incrementally and the drain has nothing left.
            # Observing all input-DMA lanes BEFORE issuing the output DMA
            # also elides the out-DMA's same-lane ordering wait (HWDGE
            # waits execute on the issuing sequencer), keeping it at the
            # one-wait encoding limit regardless of input DMA count.
            for t in sync_targets:
                nop = nc.sync.nop()
                add_dep_helper(
                    nop.ins, t.ins, sync=True, reason="spread drain waits"
                )

            # Out-DMA on the gpsimd ring: it carries no other DMAs, so the
            # only wait is the o_sb producer tick.
            late_targets.append(nc.gpsimd.dma_start(
                out=out[:], in_=o_sb[:], single_packet=True
            ))
            for t in late_targets:
                nop = nc.sync.nop()
                add_dep_helper(
                    nop.ins, t.ins, sync=True, reason="spread drain waits"
                )

    return nc


def _fold_weights(gene_idx, lda_W, lda_b, W0, b0):
    """Fold gather + per-LDA linear + W0 into Mfold [NG, C, J0] and
    bias0 [C, J0], computed in float64."""
    lda_W64 = lda_W.astype(np.float64)
    W064 = W0.astype(np.float64)
    # A[l, n, c] = sum_g [gene_idx[l,g]==n] * lda_W[l,c,g]
    A = np.zeros((L, NG, C), dtype=np.float64)
    l_rep = np.repeat(np.arange(L), G)
    np.add.at(A, (l_rep, gene_idx.ravel()), lda_W64.transpose(0, 2, 1).reshape(L * G, C))
    # Mfold[n, c, j] = sum_l A[l, n, c] * W0[j, l]
    Mfold = (W064 @ A.reshape(L, NG * C)).reshape(J0, NG, C).transpose(1, 2, 0)
    bias0 = np.einsum("lc,jl->cj", lda_b.astype(np.float64), W064) + b0.astype(
        np.float64
    )
    return Mfold, bias0


_prog_cache = {}


def _get_program(act=None, mm="f16"):
    key = ("nc", act, mm)
    if key not in _prog_cache:
        _prog_cache[key] = _build_program(act, mm)
    return _prog_cache[key]


def _round_tf32(a):
    """Round fp32 array to the TF32 grid (10-bit mantissa, RNE)."""
    u = np.ascontiguousarray(a, dtype=np.float32).view(np.uint32)
    lsb = (u >> 13) & np.uint32(1)
    u2 = (u + np.uint32(0x0FFF) + lsb) & np.uint32(0xFFFFE000)
    return u2.view(np.float32)


def _mm_convert(a, mm):
    if mm == "f32":
        return np.asarray(a, dtype=np.float32)
    if mm == "f32r":
        return _round_tf32(np.asarray(a, dtype=np.float32))
    if mm == "bf16":
        import ml_dtypes

        return np.asarray(a, dtype=np.float32).astype(ml_dtypes.bfloat16)
    if mm == "f16":
        return np.asarray(a, dtype=np.float32).astype(np.float16)
    raise ValueError(mm)


def _prepare_in_maps(X, gene_idx, lda_W, lda_b, W0, b0, W1, b1, W2, b2, mm="f16"):
    X = np.asarray(X, dtype=np.float32)
    gene_idx = np.asarray(gene_idx)
    lda_W = np.asarray(lda_W, dtype=np.float32)
    lda_b = np.asarray(lda_b, dtype=np.float32)
    W0 = np.asarray(W0, dtype=np.float32)
    b0 = np.asarray(b0, dtype=np.float32)
    W1 = np.asarray(W1, dtype=np.float32)
    b1 = np.asarray(b1, dtype=np.float32)
    W2 = np.asarray(W2, dtype=np.float32)
    b2 = np.asarray(b2, dtype=np.float32)

    Mfold, bias0 = _fold_weights(gene_idx, lda_W, lda_b, W0, b0)

    # Per-C-half M shards: Mfold columns c-major flattened, scaled by SM,
    # bias0 in row NG (met by X's 1/SX ones-row), sentinel col M0.
    mf_maps, w12_maps = [], []
    for ch in range(QC):
        cs = slice(ch * CS, (ch + 1) * CS)
        mcols = Mfold[:, cs, :].reshape(NG, M0)
        mpad = np.zeros((KP, M0P), dtype=np.float64)
        mpad[:NG, :M0] = mcols * SM
        mpad[NG, :M0] = bias0[cs, :].reshape(M0) * SM
        mpad[NG, M0] = SENT * SM  # with X's 1/SX row: h0_pre[M0,:] == SENT
        mf_maps.append(np.ascontiguousarray(
            mpad.reshape(KT, 128, M0P).transpose(1, 0, 2)))

        # Packed fp16 W1blk/W2blk [128, W12W].
        w1blk = np.zeros((M0P, M1P), dtype=np.float64)
        for c in range(CS):
            w1blk[c * J0 : (c + 1) * J0, c * J1 : (c + 1) * J1] = W1.T
        # b1 rides on the h0 sentinel channel (value SENT after gelu).
        for c in range(CS):
            w1blk[M0, c * J1 : (c + 1) * J1] = b1 / SENT
        # b2 hook: make h1 channel M1 a sentinel too.
        w1blk[M0, M1] = 1.0
        w2blk = np.zeros((M1P, CS), dtype=np.float64)
        for c in range(CS):
            w2blk[c * J1 : (c + 1) * J1, c] = W2[0]
        w2blk[M1, :] = b2[0] / SENT
        w_arr = np.zeros((128, W12W), dtype=np.float64)
        w_arr[:, : MT * M1P] = (
            w1blk.reshape(MT, 128, M1P).transpose(1, 0, 2).reshape(128, MT * M1P)
        )
        w_arr[:M1P, MT * M1P :] = w2blk
        w12_maps.append(w_arr)

    # Batch shards, transposed + 1/SX ones-row + swizzled to [128, KT, BS].
    xt_maps = []
    for bq in range(PB):
        xs = X[bq * BS : (bq + 1) * BS, :]  # [BS, NG]
        xpad = np.zeros((KP, BS), dtype=np.float64)
        xpad[:NG, :] = xs.T * SX
        xpad[NG, :] = SX
        xt_maps.append(np.ascontiguousarray(
            xpad.reshape(KT, 128, BS).transpose(1, 0, 2)))

    in_maps = []
    for core in range(N_CORES):
        bq, ch = core % PB, core // PB
        mxa = np.concatenate([mf_maps[ch], xt_maps[bq]], axis=2)
        in_maps.append({
            "mx": _mm_convert(np.ascontiguousarray(mxa), mm),
            "w12": _mm_convert(w12_maps[ch], mm),
        })
    return in_maps


def _assemble(core_outs):
    out = np.empty((B, C, 1), dtype=np.float32)
    for core in range(N_CORES):
        bq, ch = core % PB, core // PB
        o = core_outs[core]  # [CS, BS]
        out[bq * BS : (bq + 1) * BS, ch * CS : (ch + 1) * CS, 0] = o.T
    return out


MM_MODE = "f16"


def kernel(X, gene_idx, lda_W, lda_b, W0, b0, W1, b1, W2, b2, _trace=False,
           _mm=None):
    mm = _mm or MM_MODE
    in_maps = _prepare_in_maps(
        X, gene_idx, lda_W, lda_b, W0, b0, W1, b1, W2, b2, mm=mm
    )
    nc = _get_program(mm=mm)
    res = run_bass_kernel_spmd(
        nc, in_maps, core_ids=list(range(N_CORES)), trace=_trace
    )
    out = _assemble([res.results[c]["out"] for c in range(N_CORES)])
    if _trace:
        return out, res
    return out


# revision 22
# speedup vs baseline: 1.0592x; 1.0029x over previous
"""DeepSwarmLDA Trainium2 kernel (fp16 streaming version).

Math: reference computes
    Xg        = X[:, gene_idx]                        [B, L, G]
    ldas_out  = einsum('blg,lcg->bcl', Xg, lda_W) + lda_b.T
    h         = gelu(ldas_out @ W0.T + b0)            [B, C, 100]
    h         = gelu(h @ W1.T + b1)                   [B, C, 10]
    out       = h @ W2.T + b2                         [B, C, 1]

Everything up to the first gelu is linear in X, so the gather, the per-LDA
classifiers and W0 fold (on host, in float64) into one dense matrix:
    Mfold[n, (c,j)] = sum_{l,g} [gene_idx[l,g]==n] * lda_W[l,c,g] * W0[j,l]
    bias0[(c,j)]    = sum_l lda_b[l,c] * W0[j,l] + b0[j]
giving  h0 = gelu(X @ Mfold + bias0).  The remaining layers act per-c and
fold into block-diagonal matrices W1blk [(c,j),(c,k)] and W2blk [(c,k), c].

Device computes everything transposed (batch on the matmul free axis) so the
contraction dim always sits on SBUF partitions and no transposes are needed:
    h0T[(c,j), b] = gelu(Mfold_tile.T @ XT)           (bias0 via ones-row)
    h1T[(c,k), b] = gelu(W1blk.T @ h0T)               (b1 via sentinel chan)
    outT[c, b]    = W2blk.T @ h1T                     (b2 via sentinel chan)

All biases ride inside the matmuls using pad space that already exists:
  - X row NG (=2000, inside the 2048 pad) is a constant 1/SX row and
    Mfold row NG holds bias0*SM, so X@M picks up bias0 exactly.
  - Mfold pad column M0 (=500) makes h0_pre[500,:] == 8.0 exactly;
    gelu(8) == 8 to fp64 precision, so W1blk row 500 = b1/8 adds b1, and
    the same sentinel trick through W1blk col 50 adds b2 via W2blk row 50.

Matmul operands are fp16: Mfold scaled by SM=32, X by SX=1/32 (exact powers
of two, product unscaled) so every fp16 value sits far from the subnormal
range even though Mfold entries are ~5e-4. fp16 keeps tf32-level accuracy
(host sim: 3.4e-4 absmax-rel) at half the fp32 DMA bytes and full PE rate.

Sharding over 8 cores: batch split 4 ways (256 rows each) x C split 2 ways
(c 0-4 / c 5-9, so the aggregation MLP stays core-local).
"""

import numpy as np

import concourse.bass as bass
import concourse.mybir as mybir
from concourse.tile import TileContext
from concourse.tile_rust import add_dep_helper
from concourse.bass_utils import run_bass_kernel_spmd

# Problem shape (hardcoded per contract; kernel.py must be self-contained).
B, NG, L, G, C = 1024, 2000, 1000, 50, 10
J0, J1 = 100, 10

N_CORES = 8
PB, QC = 4, 2              # batch split x c split
BS = B // PB               # 256 batch rows per core
CS = C // QC               # 5 classes per core
KP = 2048                  # NG padded to 16 k-tiles of 128
KT = KP // 128             # 16
M0 = CS * J0               # 500 h0 channels per core
M0P = 512                  # padded -> 4 m-tiles of 128
MT = M0P // 128            # 4
M1 = CS * J1               # 50 h1 channels per core
M1P = 64                   # padded
F32 = mybir.dt.float32

# Operand scaling (exact powers of two; product is unscaled).
SM, SX = 32.0, 1.0 / 32.0
SENT = 8.0                 # sentinel: gelu(8) == 8 exactly in fp64/fp16

# DMA chunk sizes (k-tiles per DMA). Small first chunks so the PE starts
# early; small last chunks so the epilogue's final dependency lands early.
CHUNKS = [1, 1, 2, 2, 2, 2, 2, 2, 1, 1]
W12W = MT * M1P + CS       # 261 packed fp16 W1blk+W2blk columns

MM_DTYPES = {
    "f32": mybir.dt.float32,
    "f32r": mybir.dt.float32r,
    "bf16": mybir.dt.bfloat16,
    "f16": mybir.dt.float16,
}


def _build_program(act=None, mm="f16"):
    act = act if act is not None else mybir.ActivationFunctionType.Gelu
    mmdt = MM_DTYPES[mm]
    nc = bass.Bass()
    # DRAM layouts are pre-swizzled on host so every DMA is contiguous.
    # M-operand and X-operand interleave per k-tile in ONE tensor so each
    # chunk arrives via a single DMA (a matmul may depend on at most one
    # in-flight transfer):
    #   mx [128, KT, M0P+BS]  mx[p, k, :M0P] = Mfold[k*128+p, colshard]*SM
    #                         mx[p, k, M0P:] = X[bshard, k*128+p]*SX
    # w12 [128, W12W] fp16:   cols 0:256  W1blk swizzled
    #                         (col m*64+o = W1blk[m*128+p, o])
    #                         cols 256:261 W2blk (rows 0:64)
    mx = nc.declare_dram_parameter("mx", [128, KT, M0P + BS], mmdt, isOutput=False)
    w12 = nc.declare_dram_parameter("w12", [128, W12W], mmdt, isOutput=False)
    out = nc.declare_dram_parameter("out", [CS, BS], F32, isOutput=True)

    with TileContext(nc) as tc:
        with (
            tc.tile_pool(name="big", bufs=1) as big,
            tc.tile_pool(name="small", bufs=1) as small,
            tc.tile_pool(name="psum", bufs=1, space="PSUM") as psum,
        ):
            sync_targets = []     # observed by SP nops BEFORE the out-DMA
            late_targets = []     # observed after

            # Big operand chunks (one tile per chunk so the PE starts as
            # soon as a chunk lands). ALL triggers go on ONE ring: the 16
            # DMA queues are shared and work-conserving, so concurrent
            # triggers from different rings interleave their descriptors
            # and every chunk completes later than its wire-serial
            # position. Serial descriptor generation (~0.7us per chunk)
            # stays well ahead of the ~1.3us/chunk wire time.
            # The final k-tile (k15) is only 81 meaningful partition rows:
            # X rows 1920..1999 plus the bias ones-row at 2000; the rest is
            # zero padding. Trimming its DMA to 81 rows lands the
            # tail-critical last chunk ~0.3us earlier.
            KLAST = NG - 15 * 128 + 1  # 81

            mx_ch = []   # per k-tile: (chunk tile, chunk idx, partitions)
            c0 = 0
            for ci, csz in enumerate(CHUNKS):
                pp = KLAST if c0 + csz == KT else 128
                t = big.tile([pp, csz, M0P + BS], mmdt, name=f"mx_ch{ci}")
                sync_targets.append(
                    nc.sync.dma_start(out=t[:], in_=mx[0:pp, c0 : c0 + csz])
                )
                for j in range(csz):
                    mx_ch.append((t, j, pp))
                c0 += csz
            assert c0 == KT

            # w12 rides LAST on the same ring: the k-tile chunks are not
            # delayed by its 67KB, and it still lands ~1us before its
            # first consumer (the PE touch before the h1 matmuls).
            w12_sb = small.tile([128, W12W], mmdt, name="w12_sb")
            sync_targets.append(nc.sync.dma_start(out=w12_sb[:], in_=w12[:]))

            # PE warmup: the PE runs at 0.65/1.2 GHz until it has been
            # continuously busy for ~3us, only then at 2.4 GHz. Dep-free
            # dummy matmuls on (uninitialized) SBUF ramp it to full clock
            # during the trigger+first-transfer dead time, so the real
            # stream starts at full rate. Results land in a scratch PSUM
            # tile; every start/stop pair precedes the first real matmul
            # in PE program order, so bank clears cannot hurt real data.
            warm_sb = small.tile([128, 256], mmdt, name="warm_sb")
            nc.vector.memset(warm_sb[:], 0)
            warm_ps = psum.tile([128, 256], F32, name="warm_ps")

            def pe_warm(n):
                for _ in range(n):
                    nc.tensor.matmul(
                        warm_ps[:],
                        lhsT=warm_sb[:, 0:128],
                        rhs=warm_sb[:],
                        start=True,
                        stop=True,
                    )

            pe_warm(32)

            # h0T = Mfold.T @ XT accumulated over 16 k-tiles, 4 m-tiles.
            # One PSUM tile per m-tile: matmul start=True clears the whole
            # PSUM bank, so accumulation regions must not share banks.
            h0_ps = [
                psum.tile([128, BS], F32, name=f"h0_ps{m}") for m in range(MT)
            ]
            for k in range(KT):
                t, j, pp = mx_ch[k]
                ch = t[:, j]
                for m in range(MT):
                    nc.tensor.matmul(
                        h0_ps[m][:],
                        lhsT=ch[0:pp, m * 128 : (m + 1) * 128],
                        rhs=ch[0:pp, M0P:],
                        start=(k == 0),
                        stop=(k == KT - 1),
                    )

            # PE touch: observe the w12 lane between the big matmuls and
            # the w12-consuming h1 matmuls (keeps those at one new wait
            # each without ever stalling the in-order PE queue).
            t_ps = psum.tile([1, 1], F32, name="t_ps")
            nc.tensor.matmul(
                t_ps[:], lhsT=w12_sb[:, 0:1], rhs=w12_sb[:, 0:1],
                start=True, stop=True,
            )

            # gelu(h0), PSUM -> SBUF fp16, one slab per m-tile so each
            # pipelines against the PE's final k-tile matmuls.
            h0_sb = small.tile([128, MT, BS], mmdt, name="h0_sb")
            for m in range(MT):
                nc.scalar.activation(h0_sb[:, m], h0_ps[m][:], act)

            # h1T = W1blk.T @ h0T (contraction = 4 m-tiles of 128).
            h1_ps = psum.tile([M1P, BS], F32, name="h1_ps")
            for m in range(MT):
                nc.tensor.matmul(
                    h1_ps[:],
                    lhsT=w12_sb[:, m * M1P : (m + 1) * M1P],
                    rhs=h0_sb[:, m],
                    start=(m == 0),
                    stop=(m == MT - 1),
                )
            h1_sb = small.tile([M1P, BS], mmdt, name="h1_sb")
            nc.scalar.activation(h1_sb[:], h1_ps[:], act)

            # outT = W2blk.T @ h1T  (b2 rides on sentinel channel 50).
            o_ps = psum.tile([CS, BS], F32, name="o_ps")
            sync_targets.append(nc.tensor.matmul(
                o_ps[:],
                lhsT=w12_sb[0:M1P, MT * M1P : MT * M1P + CS],
                rhs=h1_sb[:],
                start=True,
                stop=True,
            ))
            o_sb = small.tile([CS, BS], F32, name="o_sb")
            late_targets.append(nc.vector.tensor_copy(out=o_sb[:], in_=o_ps[:]))

            # The kernel-tail drain puts a wait on every proc SP has not
            # observed, and its encoding holds only a few waits. Chain SP
            # NOPs, one sync dep each, so SP observes every DMA lane and
            # engine tick incrementally and the drain has nothing left.
            # Observing all input-DMA lanes BEFORE issuing the output DMA
            # also elides the out-DMA's same-lane ordering wait (HWDGE
            # waits execute on the issuing sequencer), keeping it at the
            # one-wait encoding limit regardless of input DMA count.
            for t in sync_targets:
                nop = nc.sync.nop()
                add_dep_helper(
                    nop.ins, t.ins, sync=True, reason="spread drain waits"
                )

            # Out-DMA on the gpsimd ring: it carries no other DMAs, so the
            # only wait is the o_sb producer tick.
            late_targets.append(nc.gpsimd.dma_start(
                out=out[:], in_=o_sb[:], single_packet=True
            ))
            for t in late_targets:
                nop = nc.sync.nop()
                add_dep_helper(
                    nop.ins, t.ins, sync=True, reason="spread drain waits"
                )

    return nc


def _fold_weights(gene_idx, lda_W, lda_b, W0, b0):
    """Fold gather + per-LDA linear + W0 into Mfold [NG, C, J0] and
    bias0 [C, J0], computed in float64."""
    lda_W64 = lda_W.astype(np.float64)
    W064 = W0.astype(np.float64)
    # A[l, n, c] = sum_g [gene_idx[l,g]==n] * lda_W[l,c,g]
    A = np.zeros((L, NG, C), dtype=np.float64)
    l_rep = np.repeat(np.arange(L), G)
    np.add.at(A, (l_rep, gene_idx.ravel()), lda_W64.transpose(0, 2, 1).reshape(L * G, C))
    # Mfold[n, c, j] = sum_l A[l, n, c] * W0[j, l]
    Mfold = (W064 @ A.reshape(L, NG * C)).reshape(J0, NG, C).transpose(1, 2, 0)
    bias0 = np.einsum("lc,jl->cj", lda_b.astype(np.float64), W064) + b0.astype(
        np.float64
    )
    return Mfold, bias0


_prog_cache = {}


def _get_program(act=None, mm="f16"):
    key = ("nc", act, mm)
    if key not in _prog_cache:
        _prog_cache[key] = _build_program(act, mm)
    return _prog_cache[key]


def _round_tf32(a):
    """Round fp32 array to the TF32 grid (10-bit mantissa, RNE)."""
    u = np.ascontiguousarray(a, dtype=np.float32).view(np.uint32)
    lsb = (u >> 13) & np.uint32(1)
    u2 = (u + np.uint32(0x0FFF) + lsb) & np.uint32(0xFFFFE000)
    return u2.view(np.float32)


def _mm_convert(a, mm):
    if mm == "f32":
        return np.asarray(a, dtype=np.float32)
    if mm == "f32r":
        return _round_tf32(np.asarray(a, dtype=np.float32))
    if mm == "bf16":
        import ml_dtypes

        return np.asarray(a, dtype=np.float32).astype(ml_dtypes.bfloat16)
    if mm == "f16":
        return np.asarray(a, dtype=np.float32).astype(np.float16)
    raise ValueError(mm)


def _prepare_in_maps(X, gene_idx, lda_W, lda_b, W0, b0, W1, b1, W2, b2, mm="f16"):
    X = np.asarray(X, dtype=np.float32)
    gene_idx = np.asarray(gene_idx)
    lda_W = np.asarray(lda_W, dtype=np.float32)
    lda_b = np.asarray(lda_b, dtype=np.float32)
    W0 = np.asarray(W0, dtype=np.float32)
    b0 = np.asarray(b0, dtype=np.float32)
    W1 = np.asarray(W1, dtype=np.float32)
    b1 = np.asarray(b1, dtype=np.float32)
    W2 = np.asarray(W2, dtype=np.float32)
    b2 = np.asarray(b2, dtype=np.float32)

    Mfold, bias0 = _fold_weights(gene_idx, lda_W, lda_b, W0, b0)

    # Per-C-half M shards: Mfold columns c-major flattened, scaled by SM,
    # bias0 in row NG (met by X's 1/SX ones-row), sentinel col M0.
    mf_maps, w12_maps = [], []
    for ch in range(QC):
        cs = slice(ch * CS, (ch + 1) * CS)
        mcols = Mfold[:, cs, :].reshape(NG, M0)
        mpad = np.zeros((KP, M0P), dtype=np.float64)
        mpad[:NG, :M0] = mcols * SM
        mpad[NG, :M0] = bias0[cs, :].reshape(M0) * SM
        mpad[NG, M0] = SENT * SM  # with X's 1/SX row: h0_pre[M0,:] == SENT
        mf_maps.append(np.ascontiguousarray(
            mpad.reshape(KT, 128, M0P).transpose(1, 0, 2)))

        # Packed fp16 W1blk/W2blk [128, W12W].
        w1blk = np.zeros((M0P, M1P), dtype=np.float64)
        for c in range(CS):
            w1blk[c * J0 : (c + 1) * J0, c * J1 : (c + 1) * J1] = W1.T
        # b1 rides on the h0 sentinel channel (value SENT after gelu).
        for c in range(CS):
            w1blk[M0, c * J1 : (c + 1) * J1] = b1 / SENT
        # b2 hook: make h1 channel M1 a sentinel too.
        w1blk[M0, M1] = 1.0
        w2blk = np.zeros((M1P, CS), dtype=np.float64)
        for c in range(CS):
            w2blk[c * J1 : (c + 1) * J1, c] = W2[0]
        w2blk[M1, :] = b2[0] / SENT
        w_arr = np.zeros((128, W12W), dtype=np.float64)
        w_arr[:, : MT * M1P] = (
            w1blk.reshape(MT, 128, M1P).transpose(1, 0, 2).reshape(128, MT * M1P)
        )
        w_arr[:M1P, MT * M1P :] = w2blk
        w12_maps.append(w_arr)

    # Batch shards, transposed + 1/SX ones-row + swizzled to [128, KT, BS].
    xt_maps = []
    for bq in range(PB):
        xs = X[bq * BS : (bq + 1) * BS, :]  # [BS, NG]
        xpad = np.zeros((KP, BS), dtype=np.float64)
        xpad[:NG, :] = xs.T * SX
        xpad[NG, :] = SX
        xt_maps.append(np.ascontiguousarray(
            xpad.reshape(KT, 128, BS).transpose(1, 0, 2)))

    in_maps = []
    for core in range(N_CORES):
        bq, ch = core % PB, core // PB
        mxa = np.concatenate([mf_maps[ch], xt_maps[bq]], axis=2)
        in_maps.append({
            "mx": _mm_convert(np.ascontiguousarray(mxa), mm),
            "w12": _mm_convert(w12_maps[ch], mm),
        })
    return in_maps


def _assemble(core_outs):
    out = np.empty((B, C, 1), dtype=np.float32)
    for core in range(N_CORES):
        bq, ch = core % PB, core // PB
        o = core_outs[core]  # [CS, BS]
        out[bq * BS : (bq + 1) * BS, ch * CS : (ch + 1) * CS, 0] = o.T
    return out


MM_MODE = "f16"


def kernel(X, gene_idx, lda_W, lda_b, W0, b0, W1, b1, W2, b2, _trace=False,
           _mm=None):
    mm = _mm or MM_MODE
    in_maps = _prepare_in_maps(
        X, gene_idx, lda_W, lda_b, W0, b0, W1, b1, W2, b2, mm=mm
    )
    nc = _get_program(mm=mm)
    res = run_bass_kernel_spmd(
        nc, in_maps, core_ids=list(range(N_CORES)), trace=_trace
    )
    out = _assemble([res.results[c]["out"] for c in range(N_CORES)])
    if _trace:
        return out, res
    return out
